# revision 30
# baseline (speedup 1.0000x reference)
"""Trainium2 Bass kernel for nn_CINLayer: out[b,d,o] = sum_{n,m} x[b,d,n]*y[b,d,m]*W[o,n*M+m].

Strategy (8-core data parallel over batch):
  Per sample s, out[o,s] = sum_k Wl[k,o] * Z[k,s] with Z[k,s] = x[s,n(k)]*y[s,m(k)].
  The 1600-term contraction is split into 13 chunks of 128 rows whose
  row->(n,m) mapping is chosen so that BOTH factor tiles of every chunk are
  host-staged replicated layouts (no on-device cross-partition movement):

    chunks 0-4  (n 0..15):  n = r//8,      m = 8c + r%8
    chunks 5-9  (n 16..31): n = 16 + r//8, m = 8(c-5) + r%8
    chunks 10-12 (n 32..39): n = 32 + r//16, m = 16(c-10) + r%16  (m>=40 -> W row zeroed)

  Only 10 distinct [128, S] factor tiles are needed per core (chunk 12's
  y-factor yT[32+r%16] coincides with y4=yT[32+r%8] on all W-valid rows):
    xg0[r]=xT[r//8], xg1[r]=xT[16+r//8], xg2[r]=xT[32+r//16],
    y0..y4[r]=yT[8q+r%8], yg2_0[r]=yT[r%16], yg2_1[r]=yT[16+r%16]
  They are staged in chunk-need order as one contiguous per-partition block
  per loop iteration, fetched as two large need-ordered DMAs. Per chunk the
  DVE does one fp16 tensor_mul (2x mode) and the PE accumulates
  out^T[o,s] (o split 128+72, s tiles of 512) in PSUM. Dummy warm-up matmuls
  run during the input-DMA head so the PE clock (HAM) is un-throttled when
  the real stream starts.
"""

import numpy as np

BS, DIM, N, M, O = 2048, 32, 40, 40, 200
NCORES = 8
S_PER_CORE = BS * DIM // NCORES  # 8192
S_TILE = 512
N_STILES_FULL = S_PER_CORE // S_TILE  # 16
NCHUNKS = 13
NTILES = 10  # staged factor tiles, need-order (see _stage_core_inputs)
W2 = 2 * S_TILE  # max samples per loop iteration (paired s-tiles)
SPLIT_A = 6  # first DMA piece: tiles 0..5 (enables chunks 0-4)
F16 = np.float16

# iteration widths: progressively narrower final iterations shrink the
# drain tail (last matmul -> last output-DMA receipt)
WIDTHS = [S_TILE] + [W2] * 7 + [S_TILE // 2, S_TILE // 4, S_TILE // 4]


def _chunk_row_to_nm(c: int, r: int):
    """Chunk c (0..12), row r (0..127) -> (n, m) or None (zero pad)."""
    if c < 5:
        return r // 8, 8 * c + r % 8
    if c < 10:
        return 16 + r // 8, 8 * (c - 5) + r % 8
    m = 16 * (c - 10) + r % 16
    if m >= M:
        return None
    return 32 + r // 16, m


def _chunk_srcs(c: int):
    """Chunk c -> (x tile idx, y tile idx) in the need-ordered staged block."""
    if c < 5:
        return 0, 1 + c
    if c < 10:
        return 6, 1 + (c - 5)
    return 7, {10: 8, 11: 9, 12: 5}[c]


def _stage_w(W: np.ndarray) -> np.ndarray:
    """W [O, N*M] f32 -> wl [128, NCHUNKS, O] f16 (lhsT layout per chunk)."""
    Wr = W.reshape(O, N, M)
    wl = np.zeros((128, NCHUNKS, O), dtype=F16)
    for c in range(NCHUNKS):
        for r in range(128):
            nm = _chunk_row_to_nm(c, r)
            if nm is not None:
                wl[r, c, :] = Wr[:, nm[0], nm[1]].astype(F16)
    return wl


def _stage_core_inputs(x_flat: np.ndarray, y_flat: np.ndarray) -> np.ndarray:
    """x_flat, y_flat [S_PER_CORE, 40] f32 -> xy [128, sum(NTILES*w)] f16.

    Need-ordered tiles, flattened per iteration block so each DMA piece is
    contiguous per partition."""
    xT = np.ascontiguousarray(x_flat.T).astype(F16)  # [40, S]
    yT = np.ascontiguousarray(y_flat.T).astype(F16)  # [40, S]
    r = np.arange(128)
    tiles = [
        xT[r // 8],        # 0: xg0   (chunks 0-4)
        yT[0 + r % 8],     # 1: y0    (chunks 0, 5)
        yT[8 + r % 8],     # 2: y1    (chunks 1, 6)
        yT[16 + r % 8],    # 3: y2    (chunks 2, 7)
        yT[24 + r % 8],    # 4: y3    (chunks 3, 8)
        yT[32 + r % 8],    # 5: y4    (chunks 4, 9, 12)
        xT[16 + r // 8],   # 6: xg1   (chunks 5-9)
        xT[32 + r // 16],  # 7: xg2   (chunks 10-12)
        yT[r % 16],        # 8: yg2_0 (chunk 10)
        yT[16 + r % 16],   # 9: yg2_1 (chunk 11)
    ]
    stk = np.stack(tiles, axis=1)  # [128, NTILES, S]
    blocks = []
    s0 = 0
    for w in WIDTHS:
        blocks.append(stk[:, :, s0 : s0 + w].reshape(128, NTILES * w))
        s0 += w
    return np.ascontiguousarray(np.concatenate(blocks, axis=1))


def build_nc(debug: bool = False):
    """Build the per-core Bass/Tile module. Returns nc."""
    import concourse.bass as bass
    import concourse.tile as tile
    from concourse import bacc, mybir

    f16 = mybir.dt.float16
    f32 = mybir.dt.float32
    s_len = sum(WIDTHS)
    flat_len = NTILES * s_len
    N_WARM = 56

    nc = bacc.Bacc("TRN2", target_bir_lowering=False, debug=debug)

    xy_d = nc.dram_tensor("xy", [128, flat_len], f16, kind="ExternalInput")
    wl_d = nc.dram_tensor("wl", [128, NCHUNKS, O], f16, kind="ExternalInput")
    out_d = nc.dram_tensor("outt", [O, s_len], f16, kind="ExternalOutput")

    with tile.TileContext(nc) as tc:
        with (
            tc.tile_pool(name="wpool", bufs=1) as wpool,
            tc.tile_pool(name="inp", bufs=4) as inp,
            tc.tile_pool(name="zp", bufs=8) as zp,
            tc.tile_pool(name="outp", bufs=2) as outp,
            tc.tile_pool(name="ps", bufs=2, space=bass.MemorySpace.PSUM) as psp,
        ):
            wl_sb = wpool.tile([128, NCHUNKS, O], f16)
            # scalar queue so wl streams concurrently with the sync-queue
            # xy tiles; split so the first chunks' weights (which gate the
            # first LDWEIGHTS) land early
            nc.scalar.dma_start(wl_sb[:, 0:4, :], wl_d[:, 0:4, :])
            nc.scalar.dma_start(wl_sb[:, 4:NCHUNKS, :], wl_d[:, 4:NCHUNKS, :])

            # PE warmup: dummy matmuls while input DMAs land, so HAM has
            # un-throttled the clock (1.2->2.4 GHz) before the real stream;
            # sized to keep PE busy right up to first-data-ready (a gap would
            # let HAM re-throttle)
            warm_sb = wpool.tile([128, 128], f16)
            nc.gpsimd.memset(warm_sb[:], 0)
            warm_ps = psp.tile([128, S_TILE], f32, tag="psA0")
            for _ in range(N_WARM):
                nc.tensor.matmul(
                    warm_ps[:, 0:128], warm_sb[:], warm_sb[:],
                    start=True, stop=True,
                )

            s0 = 0
            off = 0
            for it, w in enumerate(WIDTHS):
                sl = bass.ds(s0, w)
                xy_t = inp.tile([128, NTILES * W2], f16)
                # need-ordered pieces, all on the sync queue (FIFO per queue
                # -> completion tracks chunk need order). First iteration:
                # per-tile, so the supply cadence matches the chunk demand
                # cadence from the very first matmul; later: 2-tile pieces.
                pieces = [(2 * p, 2 * p + 2) for p in range(5)]
                for lo, hi in pieces:
                    nc.sync.dma_start(
                        xy_t[:, lo * w : hi * w],
                        xy_d[:, off + lo * w : off + hi * w],
                    )

                # h-tiles of <=512 samples (PSUM bank limit per matmul)
                hws = []
                h0 = 0
                while h0 < w:
                    hws.append((h0, min(S_TILE, w - h0)))
                    h0 += S_TILE
                ps = []
                for h in range(len(hws)):
                    ps.append(
                        psp.tile([128, S_TILE], f32, tag=f"psA{h}", name=f"psA{h}")
                    )
                    ps.append(
                        psp.tile([72, S_TILE], f32, tag=f"psB{h}", name=f"psB{h}")
                    )
                for c in range(NCHUNKS):
                    xi, yi = _chunk_srcs(c)
                    z = zp.tile([128, W2], f16)
                    nc.vector.tensor_mul(
                        z[:, 0:w],
                        xy_t[:, yi * w : yi * w + w],
                        xy_t[:, xi * w : xi * w + w],
                    )
                    first, last = c == 0, c == NCHUNKS - 1
                    for h, (hs0, hw) in enumerate(hws):
                        zh = z[:, hs0 : hs0 + hw]
                        # full-tile PSUM writes ([:]) lower to a cheaper AP
                        # than slices: only slice the sub-512 drain tiles
                        pA = ps[2 * h][:] if hw == S_TILE else ps[2 * h][:, 0:hw]
                        pB = (
                            ps[2 * h + 1][:]
                            if hw == S_TILE
                            else ps[2 * h + 1][:, 0:hw]
                        )
                        nc.tensor.matmul(
                            pA, wl_sb[:, c, 0:128], zh,
                            start=first, stop=last,
                        )
                        nc.tensor.matmul(
                            pB, wl_sb[:, c, 128:200], zh,
                            start=first, stop=last,
                        )

                oA = outp.tile([128, W2], f16, tag="oA")
                oB = outp.tile([72, W2], f16, tag="oB")
                for h, (hs0, hw) in enumerate(hws):
                    pA = ps[2 * h][:] if hw == S_TILE else ps[2 * h][:, 0:hw]
                    pB = (
                        ps[2 * h + 1][:]
                        if hw == S_TILE
                        else ps[2 * h + 1][:, 0:hw]
                    )
                    nc.scalar.copy(oA[:, hs0 : hs0 + hw], pA)
                    nc.vector.tensor_copy(oB[:, hs0 : hs0 + hw], pB)
                nc.sync.dma_start(out_d[0:128, sl], oA[:, 0:w])
                nc.scalar.dma_start(out_d[128:200, sl], oB[:, 0:w])
                s0 += w
                off += NTILES * w

    nc.compile()
    return nc


def prepare(x: np.ndarray, y: np.ndarray, W: np.ndarray):
    """Stage full inputs -> (nc, per-core input maps)."""
    assert x.shape == (BS, DIM, N) and y.shape == (BS, DIM, M)
    assert W.shape == (O, N * M)

    wl = _stage_w(W)
    x_cores = x.reshape(NCORES, S_PER_CORE, N)
    y_cores = y.reshape(NCORES, S_PER_CORE, M)
    in_maps = []
    for i in range(NCORES):
        xy = _stage_core_inputs(x_cores[i], y_cores[i])
        in_maps.append({"xy": xy, "wl": wl})
    nc = build_nc()
    return nc, in_maps


def collect(res) -> np.ndarray:
    outs = []
    for i in range(NCORES):
        outt = res.results[i]["outt"]  # [O, S_PER_CORE] f16
        outs.append(outt.T.astype(np.float32))
    return np.concatenate(outs, axis=0).reshape(BS, DIM, O)


def kernel(x: np.ndarray, y: np.ndarray, W: np.ndarray) -> np.ndarray:
    from concourse.bass_utils import run_bass_kernel_spmd

    nc, in_maps = prepare(x, y, W)
    res = run_bass_kernel_spmd(nc, in_maps, core_ids=list(range(NCORES)))
    return collect(res)


if __name__ == "__main__":
    xs = np.random.randn(BS, DIM, N).astype(np.float32)
    ys = np.random.randn(BS, DIM, M).astype(np.float32)
    Ws = (np.random.randn(O, N * M) * (1.0 / np.sqrt(N * M))).astype(np.float32)
    out = kernel(xs, ys, Ws)
    print(out.shape, out.dtype)
